# revision 41
# baseline (speedup 1.0000x reference)
"""Trainium2 Bass/Tile kernel for nn_Attention_50242527428847.

Computation (per batch element b, one NeuronCore each):
    dec[t,e]   = sum_h decoder_states[t,b,h] * W[e,h]            (projection)
    p[t,s,e]   = exp(dec[t,e] * encoder_states[s,b,e])
    denom[t,e] = sum_s p[t,s,e]
    wsum[t,s]  = sum_e p[t,s,e] / denom[t,e]
    out[t,b,d] = sum_s wsum[t,s] * encoder_inputs[s,b,d]

Design (v3 "diag-staircase"): the broadcast multiply dec[e,t]*enc[e,s] -- the
single biggest elementwise cost -- runs on the otherwise-idle PE array as
diag(dec[:,t]) @ encT, one 128-col fp16 matmul per (ce, t).  The diagonal
stationaries are materialized via a DRAM staircase:

  - `bd` is a zeros ExternalInput, [CE*4 slabs, 512K elems fp16].  After the
    projection, dec rows are DMA-scattered into slab (ce,q) at per-row flat
    offsets k*4128 + t (legal strides in DRAM address space), so slab rows
    read back at pitch 4096 place dec[k, 32k:32k+32] on row k -- and an SBUF
    AP [[1,128],[32,128]] at offset t is then exactly diag(dec[:, 32q+t]).
    The zeros never have to be rewritten; only the 32 diag values per row
    are refreshed each run.

  - Scores land in PSUM fp32 (full precision, free), ACT exps them straight
    out of PSUM into bf16 p tiles in SBUF (bf16 needed for e^{+-17} range).

Remaining engine split: ACT does all exps (the bottleneck, ~63us); DVE does
the denominator tree-adds (bf16 2x) + reciprocal; Pool takes the tree tails;
PE does projection, scores, per-t wsum N=1 matmuls (accumulated per-ce in a
shared PSUM strip, then DVE-added into an SBUF fp32 accumulator), and the
final out = wsumT.T @ enc_in.
"""

import numpy as np
from contextlib import ExitStack

import concourse.bass as bass
import concourse.bacc as bacc
import concourse.tile as tile
from concourse import mybir
from concourse.bass_utils import run_bass_kernel_spmd

TD, TE, B = 128, 128, 8
E, H, D = 512, 1024, 256
P = 128
CE = E // P          # 4 e-chunks
CH = H // P          # 8 h-chunks
Q = 4                # t-quarters per ce (32 t each)
TQ = TD // Q
SLAB = P * TQ * P    # 524288 elems per (ce,q) staircase slab
PITCH = TQ * P + TQ  # 4128: flat row pitch of the staircase write

_F32 = mybir.dt.float32
_F16 = mybir.dt.float16
_BF16 = mybir.dt.bfloat16
_CACHE = {}

# exp groups per quarter: t-sizes of the ACT instructions / PSUM ring tiles
GRP = (12, 12, 8)


def _ap(ap_in, offset_elems, dims):
    return bass.AP(tensor=ap_in.tensor, offset=ap_in.offset + offset_elems,
                   ap=[ap_in.ap[0]] + dims)


def _kernel_body(ctx, tc, out_ap, wt_ap, dt_ap, et_ap, ei_ap, bd_ap):
    nc = tc.nc
    AF = mybir.ActivationFunctionType

    singles = ctx.enter_context(tc.tile_pool(name="singles", bufs=1))
    b1_pool = ctx.enter_context(tc.tile_pool(name="bq1", bufs=4))
    p_pool = ctx.enter_context(tc.tile_pool(name="p", bufs=3))
    tr_pool = ctx.enter_context(tc.tile_pool(name="tr", bufs=2))
    psum_sc = ctx.enter_context(tc.tile_pool(name="psc", bufs=2, space="PSUM"))
    psum_m = ctx.enter_context(tc.tile_pool(name="pm", bufs=1, space="PSUM"))

    # ---- input DMAs (fp16, host-transposed) on both HW-DGE rings
    dt_sb = singles.tile([P, CH, TD], _F16)      # decoder^T [h_local, hc, t]
    dt_r = dt_ap.rearrange("(c p) t -> p c t", p=P)
    nc.sync.dma_start(out=dt_sb[:], in_=dt_r[:])
    wt_sb = singles.tile([P, CH, E], _F16)       # W^T [h_local, hc, e]
    wt_r = wt_ap.rearrange("(c p) e -> p c e", p=P)
    et_sb = singles.tile([P, CE, TE], _F16)      # enc^T [e_local, ce, s]
    nc.scalar.dma_start(out=wt_sb[:, :, 0:P], in_=wt_r[:, :, 0:P])
    nc.scalar.dma_start(out=wt_sb[:, :, P:], in_=wt_r[:, :, P:])
    nc.scalar.dma_start(out=et_sb[:], in_=et_ap.rearrange("(c p) s -> p c s", p=P))
    ei_sb = singles.tile([P, D], _F16)           # enc_in [s, d]
    nc.scalar.dma_start(out=ei_sb[:], in_=ei_ap)

    misc = psum_m.tile([P, 512], _F32)           # wps | dps | ops slices
    wps = misc[:, 0:128]
    cone = nc.const_aps.scalar_like(1.0, misc[:, 0:1])
    for _pk in range(24):
        nc.tensor.matmul(misc[0:1, _pk:_pk + 1], lhsT=cone[0:1, :],
                         rhs=cone[0:1, :], start=True, stop=True)
    ops = misc[:, 256:512]
    dps = [misc[:, 128:256], misc[:, 256:384]]   # ops region reused early

    # ---- B fills: slab (ce,q) rows (pitch TQ*P) -> SBUF diag tiles.
    # Staircase/fill DMAs ride the SP + Pool DGE rings only, so the ACT
    # ring never head-of-line blocks the exps behind a 1MB fill.
    b_tiles = {}

    def emit_stair(ce):
        # all 4 quarter-slabs of this ce in one DMA: dst (k, q, t) order
        dst = bass.AP(tensor=bd_ap.tensor,
                      offset=bd_ap.offset + ce * Q * SLAB,
                      ap=[[PITCH, P], [SLAB, Q], [1, TQ]])
        dsl = dec16[:, ce, :]
        src = bass.AP(tensor=dsl.tensor, offset=dsl.offset,
                      ap=[dsl.ap[0], [TQ, Q], [1, TQ]])
        _r = nc.sync if ce % 2 == 0 else nc.gpsimd
        _r.dma_start(out=dst, in_=src)

    def emit_bfill1(ce, q):
        # single-quarter diag-tile fill
        bt = b1_pool.tile([P, TQ * P], _F16, name="bq1")
        b_tiles[(ce, q)] = bt
        src = bass.AP(tensor=bd_ap.tensor,
                      offset=bd_ap.offset + (ce * Q + q) * SLAB,
                      ap=[[TQ * P, P], [1, TQ * P]])
        _r = nc.sync if (ce * Q + q) % 2 == 0 else nc.gpsimd
        _r.dma_start(out=bt[:], in_=src)

    hyb_p = {}

    def emit_hybrid0():
        # ce0 q0/q1: broadcast mult on the (early-idle) DVE straight from
        # SBUF, skipping the staircase round-trip; emitted inside the proj
        # loop so only dec16(0)'s copy precedes it in DVE program order
        p_t = p_pool.tile([P, TD, TE], _BF16, name="p")
        hyb_p[0] = p_t
        for qr in range(2):
            t0q = qr * TQ
            sc0 = singles.tile([P, TQ, TE], _F16, name=f"sc0_{qr}")
            eslice = et_sb[:, 0, :]
            b0 = t0q
            for tb in ((8, 8, 16) if qr == 0 else (16, 16)):
                dslice = dec16[:, 0, b0:b0 + tb]
                dec_b = bass.AP(tensor=dslice.tensor, offset=dslice.offset,
                                ap=[dslice.ap[0], dslice.ap[1], [0, TE]])
                enc_b = bass.AP(tensor=eslice.tensor, offset=eslice.offset,
                                ap=[eslice.ap[0], [0, tb], eslice.ap[1]])
                sl0 = slice(b0 - t0q, b0 - t0q + tb)
                nc.vector.tensor_mul(sc0[:, sl0, :], enc_b, dec_b)
                nc.scalar.activation(
                    out=p_t[:, b0:b0 + tb, :], in_=sc0[:, sl0, :],
                    func=AF.Exp)
                b0 += tb


    # ---- projection: dec[e_local, ce, t] = sum_h W^T[h, e] * D^T[h, t],
    # double-buffered PSUM; staircase writes + fills issued per ce ASAP
    dec16 = singles.tile([P, CE, TD], _F16)
    for ce in range(CE):
        for c in range(CH):
            nc.tensor.matmul(
                dps[ce % 2],
                lhsT=wt_sb[:, c, ce * P:(ce + 1) * P],
                rhs=dt_sb[:, c, :],
                start=(c == 0),
                stop=(c == CH - 1),
            )
        nc.vector.tensor_copy(dec16[:, ce, :], dps[ce % 2])
        emit_stair(ce)
        if ce == 0:
            for q in range(2, Q):
                emit_bfill1(0, q)
            emit_hybrid0()
        elif ce == 1:
            emit_bfill1(1, 0)
            emit_bfill1(1, 1)

    den16 = singles.tile([P, CE, TD], _BF16)
    den32 = singles.tile([P, CE, TD], _F32)
    r32 = singles.tile([P, CE, TD], _F32)
    r16 = singles.tile([P, CE, TD], _BF16)
    ws_acc = singles.tile([P, TD], _F32)
    p_ts = {}

    def emit_scores_exp_tree(ce, hooks=None):
        if ce in hyb_p:
            p_t = hyb_p[ce]
        else:
            p_t = p_pool.tile([P, TD, TE], _BF16, name="p")
        p_ts[ce] = p_t
        tr_t = tr_pool.tile([P, TD, TE // 2], _BF16, name="tr")
        for qr in range(Q):
            if hooks and qr in hooks:
                for fn in hooks[qr]:
                    fn()
            t0q = qr * TQ
            if ce == 0 and qr < 2:
                pass    # produced by emit_hybrid0 in the projection loop
            else:
                bt = b_tiles[(ce, qr)]
                bt_all = bt[:]
                tl = 0
                for gsz in GRP:
                    # scores for this group on PE: diag(dec) @ encT per t
                    sc_ps = psum_sc.tile([P, GRP[0], TE], _F32, name="sc")
                    for g in range(gsz):
                        lhs = _ap(bt_all, tl + g, [[TQ, P]])
                        nc.tensor.matmul(
                            sc_ps[:, g, :],
                            lhsT=lhs,
                            rhs=et_sb[:, ce, :],
                            start=True, stop=True,
                        )
                    # exp straight out of PSUM into bf16 p
                    t0 = t0q + tl
                    nc.scalar.activation(
                        out=p_t[:, t0:t0 + gsz, :], in_=sc_ps[:, 0:gsz, :],
                        func=AF.Exp)
                    tl += gsz
            # denominator tree for the quarter (l1-l4 DVE bf16 2x, tail
            # Pool mid-kernel / DVE for the latency-critical last quarter).
            # l1 runs in two pieces so most of it overlaps the last exp.
            sl = slice(t0q, t0q + TQ)
            sla = slice(t0q, t0q + GRP[0] + GRP[1])
            slb = slice(t0q + GRP[0] + GRP[1], t0q + TQ)
            nc.vector.tensor_add(tr_t[:, sla, 0:64], p_t[:, sla, 0:64],
                                 p_t[:, sla, 64:128])
            nc.vector.tensor_add(tr_t[:, slb, 0:64], p_t[:, slb, 0:64],
                                 p_t[:, slb, 64:128])
            nc.vector.tensor_add(tr_t[:, sl, 0:32], tr_t[:, sl, 0:32],
                                 tr_t[:, sl, 32:64])
            nc.vector.tensor_add(tr_t[:, sl, 0:16], tr_t[:, sl, 0:16],
                                 tr_t[:, sl, 16:32])
            nc.vector.tensor_add(tr_t[:, sl, 0:8], tr_t[:, sl, 0:8],
                                 tr_t[:, sl, 8:16])
            tail_eng = nc.vector if (ce == CE - 1 and qr == Q - 1) else nc.gpsimd
            tail_eng.tensor_add(tr_t[:, sl, 0:4], tr_t[:, sl, 0:4],
                                tr_t[:, sl, 4:8])
            tail_eng.tensor_add(tr_t[:, sl, 0:2], tr_t[:, sl, 0:2],
                                tr_t[:, sl, 2:4])
            tail_eng.tensor_add(den16[:, ce, sl], tr_t[:, sl, 0:1],
                                tr_t[:, sl, 1:2])

    def emit_post(ce, half=None):
        # r = 1/denom in fp32, bf16 copy for the wsum matmul rhs
        sl = slice(0, TD) if half is None else slice(half * 64, half * 64 + 64)
        nc.vector.tensor_copy(den32[:, ce, sl], den16[:, ce, sl])
        nc.vector.reciprocal_approx_fast(r32[:, ce, sl], den32[:, ce, sl])
        nc.vector.tensor_copy(r16[:, ce, sl], r32[:, ce, sl])

    def emit_wsum(ce, half=None):
        # wsum_T[s, t] += p[e, t, :].T @ r[e, t] (N=1 matmuls, ~free)
        p_t = p_ts[ce]
        sl = slice(0, TD) if half is None else slice(half * 64, half * 64 + 64)
        for t in range(sl.start, sl.stop):
            nc.tensor.matmul(
                wps[:, t:t + 1],
                lhsT=p_t[:, t, :],
                rhs=r16[:, ce, t:t + 1],
                start=True, stop=True,
            )
        if ce == 0:
            nc.vector.tensor_copy(ws_acc[:, sl], wps[:, sl])
        else:
            nc.vector.tensor_add(ws_acc[:, sl], ws_acc[:, sl], wps[:, sl])

    wsumT = singles.tile([P, TD], _F16)
    out_sb = singles.tile([P, D], _F32)

    def emit_final(half):
        t0 = half * 64
        sl = slice(t0, t0 + 64)
        nc.vector.tensor_copy(wsumT[:, sl], ws_acc[:, sl])
        nc.tensor.matmul(ops[t0:t0 + 64, :], lhsT=wsumT[:, sl],
                         rhs=ei_sb[:], start=True, stop=True,
                         tile_position=(0, t0))
        nc.vector.tensor_copy(out_sb[t0:t0 + 64, :], ops[t0:t0 + 64, :])
        nc.sync.dma_start(out=out_ap[t0:t0 + 64, :], in_=out_sb[t0:t0 + 64, :])

    # ---- software-pipelined emission: PE runs scores ahead; wsum(ce) sits
    # two ce behind so it never stalls the PE in front of ACT's exps.  The
    # tail (ce3) interleaves the remaining posts into its tree quarters.
    emit_bfill1(1, 2)
    emit_bfill1(1, 3)
    emit_scores_exp_tree(0)
    for q in range(Q):
        emit_bfill1(2, q)
    emit_scores_exp_tree(1)
    emit_post(0)
    for q in range(Q):
        emit_bfill1(3, q)
    emit_scores_exp_tree(2)
    emit_post(1)
    emit_wsum(0)
    emit_wsum(1)
    emit_scores_exp_tree(3, hooks={
        2: [lambda: emit_post(2)],
        3: [lambda: emit_post(3, 0)],
    })
    emit_wsum(2)
    emit_wsum(3, 0)
    emit_final(0)
    emit_post(3, 1)
    emit_wsum(3, 1)
    emit_final(1)


def build_program():
    if "nc" in _CACHE:
        return _CACHE["nc"]
    nc = bacc.Bacc("TRN2", target_bir_lowering=False, debug=False, num_devices=B)
    wt = nc.dram_tensor("wt", [H, E], _F16, kind="ExternalInput").ap()
    dt = nc.dram_tensor("dt", [H, TD], _F16, kind="ExternalInput").ap()
    et = nc.dram_tensor("et", [E, TE], _F16, kind="ExternalInput").ap()
    ei = nc.dram_tensor("ei", [TE, D], _F16, kind="ExternalInput").ap()
    bd = nc.dram_tensor("bd", [CE * Q * SLAB], _F16, kind="ExternalInput").ap()
    out = nc.dram_tensor("out", [TD, D], _F32, kind="ExternalOutput").ap()
    with tile.TileContext(nc) as tc:
        with ExitStack() as ctx:
            _kernel_body(ctx, tc, out, wt, dt, et, ei, bd)
    nc.compile()
    _CACHE["nc"] = nc
    return nc


_BD_ZEROS = None


def make_in_maps(encoder_inputs, encoder_states, decoder_states, W):
    global _BD_ZEROS
    if _BD_ZEROS is None:
        _BD_ZEROS = np.zeros(CE * Q * SLAB, dtype=np.float16)
    wt_np = np.ascontiguousarray(W.T).astype(np.float16)          # (H, E)
    in_maps = []
    for b in range(B):
        in_maps.append({
            "wt": wt_np,
            "dt": np.ascontiguousarray(decoder_states[:, b, :].T).astype(np.float16),
            "et": np.ascontiguousarray(encoder_states[:, b, :].T).astype(np.float16),
            "ei": np.ascontiguousarray(encoder_inputs[:, b, :]).astype(np.float16),
            "bd": _BD_ZEROS,
        })
    return in_maps


def run_on_hw(in_maps, **kwargs):
    nc = build_program()
    return run_bass_kernel_spmd(nc, in_maps, list(range(B)), **kwargs)


def kernel(**inputs):
    encoder_inputs = np.asarray(inputs["encoder_inputs"], dtype=np.float32)
    encoder_states = np.asarray(inputs["encoder_states"], dtype=np.float32)
    decoder_states = np.asarray(inputs["decoder_states"], dtype=np.float32)
    W = np.asarray(inputs["W"], dtype=np.float32)
    in_maps = make_in_maps(encoder_inputs, encoder_states, decoder_states, W)
    res = run_on_hw(in_maps)
    out = np.stack([res.results[b]["out"] for b in range(B)], axis=1)
    return np.ascontiguousarray(out.astype(np.float32))


# revision 42
# speedup vs baseline: 1.0267x; 1.0267x over previous
"""Trainium2 Bass/Tile kernel for nn_Attention_50242527428847.

Computation (per batch element b, one NeuronCore each):
    dec[t,e]   = sum_h decoder_states[t,b,h] * W[e,h]            (projection)
    p[t,s,e]   = exp(dec[t,e] * encoder_states[s,b,e])
    denom[t,e] = sum_s p[t,s,e]
    wsum[t,s]  = sum_e p[t,s,e] / denom[t,e]
    out[t,b,d] = sum_s wsum[t,s] * encoder_inputs[s,b,d]

Design (v3 "diag-staircase"): the broadcast multiply dec[e,t]*enc[e,s] -- the
single biggest elementwise cost -- runs on the otherwise-idle PE array as
diag(dec[:,t]) @ encT, one 128-col fp16 matmul per (ce, t).  The diagonal
stationaries are materialized via a DRAM staircase:

  - `bd` is a zeros ExternalInput, [CE*4 slabs, 512K elems fp16].  After the
    projection, dec rows are DMA-scattered into slab (ce,q) at per-row flat
    offsets k*4128 + t (legal strides in DRAM address space), so slab rows
    read back at pitch 4096 place dec[k, 32k:32k+32] on row k -- and an SBUF
    AP [[1,128],[32,128]] at offset t is then exactly diag(dec[:, 32q+t]).
    The zeros never have to be rewritten; only the 32 diag values per row
    are refreshed each run.

  - Scores land in PSUM fp32 (full precision, free), ACT exps them straight
    out of PSUM into bf16 p tiles in SBUF (bf16 needed for e^{+-17} range).

Remaining engine split: ACT does all exps (the bottleneck, ~63us); DVE does
the denominator tree-adds (bf16 2x) + reciprocal; Pool takes the tree tails;
PE does projection, scores, per-t wsum N=1 matmuls (accumulated per-ce in a
shared PSUM strip, then DVE-added into an SBUF fp32 accumulator), and the
final out = wsumT.T @ enc_in.
"""

import numpy as np
from contextlib import ExitStack

import concourse.bass as bass
import concourse.bacc as bacc
import concourse.tile as tile
from concourse import mybir
from concourse.bass_utils import run_bass_kernel_spmd

TD, TE, B = 128, 128, 8
E, H, D = 512, 1024, 256
P = 128
CE = E // P          # 4 e-chunks
CH = H // P          # 8 h-chunks
Q = 4                # t-quarters per ce (32 t each)
TQ = TD // Q
SLAB = P * TQ * P    # 524288 elems per (ce,q) staircase slab
PITCH = TQ * P + TQ  # 4128: flat row pitch of the staircase write

_F32 = mybir.dt.float32
_F16 = mybir.dt.float16
_BF16 = mybir.dt.bfloat16
_CACHE = {}

# exp groups per quarter: t-sizes of the ACT instructions / PSUM ring tiles
GRP = (12, 12, 8)


def _ap(ap_in, offset_elems, dims):
    return bass.AP(tensor=ap_in.tensor, offset=ap_in.offset + offset_elems,
                   ap=[ap_in.ap[0]] + dims)


def _kernel_body(ctx, tc, out_ap, wt_ap, dt_ap, et_ap, ei_ap, bd_ap):
    nc = tc.nc
    AF = mybir.ActivationFunctionType

    singles = ctx.enter_context(tc.tile_pool(name="singles", bufs=1))
    b1_pool = ctx.enter_context(tc.tile_pool(name="bq1", bufs=4))
    p_pool = ctx.enter_context(tc.tile_pool(name="p", bufs=3))
    tr_pool = ctx.enter_context(tc.tile_pool(name="tr", bufs=2))
    psum_sc = ctx.enter_context(tc.tile_pool(name="psc", bufs=2, space="PSUM"))
    psum_m = ctx.enter_context(tc.tile_pool(name="pm", bufs=1, space="PSUM"))

    # ---- input DMAs (fp16, host-transposed) on both HW-DGE rings
    dt_sb = singles.tile([P, CH, TD], _F16)      # decoder^T [h_local, hc, t]
    dt_r = dt_ap.rearrange("(c p) t -> p c t", p=P)
    nc.sync.dma_start(out=dt_sb[:], in_=dt_r[:])
    wt_sb = singles.tile([P, CH, E], _F16)       # W^T [h_local, hc, e]
    wt_r = wt_ap.rearrange("(c p) e -> p c e", p=P)
    et_sb = singles.tile([P, CE, TE], _F16)      # enc^T [e_local, ce, s]
    nc.scalar.dma_start(out=wt_sb[:, :, 0:P], in_=wt_r[:, :, 0:P])
    nc.scalar.dma_start(out=wt_sb[:, :, P:], in_=wt_r[:, :, P:])
    nc.scalar.dma_start(out=et_sb[:], in_=et_ap.rearrange("(c p) s -> p c s", p=P))
    ei_sb = singles.tile([P, D], _F16)           # enc_in [s, d]
    nc.scalar.dma_start(out=ei_sb[:], in_=ei_ap)

    misc = psum_m.tile([P, 512], _F32)           # wps | dps | ops slices
    wps = misc[:, 0:128]
    cone = nc.const_aps.scalar_like(1.0, misc[:, 0:1])
    for _pk in range(24):
        nc.tensor.matmul(misc[0:1, _pk:_pk + 1], lhsT=cone[0:1, :],
                         rhs=cone[0:1, :], start=True, stop=True)
    ops = misc[:, 256:512]
    dps = [misc[:, 128:256], misc[:, 256:384]]   # ops region reused early

    # ---- B fills: slab (ce,q) rows (pitch TQ*P) -> SBUF diag tiles.
    # Staircase/fill DMAs ride the SP + Pool DGE rings only, so the ACT
    # ring never head-of-line blocks the exps behind a 1MB fill.
    b_tiles = {}

    def emit_stair(ce):
        # all 4 quarter-slabs of this ce in one DMA: dst (k, q, t) order
        dst = bass.AP(tensor=bd_ap.tensor,
                      offset=bd_ap.offset + ce * Q * SLAB,
                      ap=[[PITCH, P], [SLAB, Q], [1, TQ]])
        dsl = dec16[:, ce, :]
        src = bass.AP(tensor=dsl.tensor, offset=dsl.offset,
                      ap=[dsl.ap[0], [TQ, Q], [1, TQ]])
        _r = nc.sync if ce % 2 == 0 else nc.gpsimd
        _r.dma_start(out=dst, in_=src)

    def emit_bfill1(ce, q):
        # single-quarter diag-tile fill
        bt = b1_pool.tile([P, TQ * P], _F16, name="bq1")
        b_tiles[(ce, q)] = bt
        src = bass.AP(tensor=bd_ap.tensor,
                      offset=bd_ap.offset + (ce * Q + q) * SLAB,
                      ap=[[TQ * P, P], [1, TQ * P]])
        _r = nc.sync if (ce * Q + q) % 2 == 0 else nc.gpsimd
        _r.dma_start(out=bt[:], in_=src)

    hyb_p = {}

    def emit_hybrid0():
        # ce0 q0/q1: broadcast mult on the (early-idle) DVE straight from
        # SBUF, skipping the staircase round-trip; emitted inside the proj
        # loop so only dec16(0)'s copy precedes it in DVE program order
        p_t = p_pool.tile([P, TD, TE], _BF16, name="p")
        hyb_p[0] = p_t
        for qr in range(2):
            t0q = qr * TQ
            sc0 = singles.tile([P, TQ, TE], _F16, name=f"sc0_{qr}")
            eslice = et_sb[:, 0, :]
            b0 = t0q
            for tb in (16, 16):
                dslice = dec16[:, 0, b0:b0 + tb]
                dec_b = bass.AP(tensor=dslice.tensor, offset=dslice.offset,
                                ap=[dslice.ap[0], dslice.ap[1], [0, TE]])
                enc_b = bass.AP(tensor=eslice.tensor, offset=eslice.offset,
                                ap=[eslice.ap[0], [0, tb], eslice.ap[1]])
                sl0 = slice(b0 - t0q, b0 - t0q + tb)
                nc.vector.tensor_mul(sc0[:, sl0, :], enc_b, dec_b)
                nc.scalar.activation(
                    out=p_t[:, b0:b0 + tb, :], in_=sc0[:, sl0, :],
                    func=AF.Exp)
                b0 += tb


    # ---- projection: dec[e_local, ce, t] = sum_h W^T[h, e] * D^T[h, t],
    # double-buffered PSUM; staircase writes + fills issued per ce ASAP
    dec16 = singles.tile([P, CE, TD], _F16)
    for ce in range(CE):
        for c in range(CH):
            nc.tensor.matmul(
                dps[ce % 2],
                lhsT=wt_sb[:, c, ce * P:(ce + 1) * P],
                rhs=dt_sb[:, c, :],
                start=(c == 0),
                stop=(c == CH - 1),
            )
        nc.vector.tensor_copy(dec16[:, ce, :], dps[ce % 2])
        emit_stair(ce)
        if ce == 0:
            for q in range(2, Q):
                emit_bfill1(0, q)
            emit_hybrid0()
        elif ce == 1:
            emit_bfill1(1, 0)
            emit_bfill1(1, 1)

    den16 = singles.tile([P, CE, TD], _BF16)
    den32 = singles.tile([P, CE, TD], _F32)
    r32 = singles.tile([P, CE, TD], _F32)
    r16 = singles.tile([P, CE, TD], _BF16)
    ws_acc = singles.tile([P, TD], _F32)
    p_ts = {}

    def emit_scores_exp_tree(ce, hooks=None):
        if ce in hyb_p:
            p_t = hyb_p[ce]
        else:
            p_t = p_pool.tile([P, TD, TE], _BF16, name="p")
        p_ts[ce] = p_t
        tr_t = tr_pool.tile([P, TD, TE // 2], _BF16, name="tr")
        for qr in range(Q):
            if hooks and qr in hooks:
                for fn in hooks[qr]:
                    fn()
            t0q = qr * TQ
            if ce == 0 and qr < 2:
                pass    # produced by emit_hybrid0 in the projection loop
            else:
                bt = b_tiles[(ce, qr)]
                bt_all = bt[:]
                tl = 0
                for gsz in GRP:
                    # scores for this group on PE: diag(dec) @ encT per t
                    sc_ps = psum_sc.tile([P, GRP[0], TE], _F32, name="sc")
                    for g in range(gsz):
                        lhs = _ap(bt_all, tl + g, [[TQ, P]])
                        nc.tensor.matmul(
                            sc_ps[:, g, :],
                            lhsT=lhs,
                            rhs=et_sb[:, ce, :],
                            start=True, stop=True,
                        )
                    # exp straight out of PSUM into bf16 p
                    t0 = t0q + tl
                    nc.scalar.activation(
                        out=p_t[:, t0:t0 + gsz, :], in_=sc_ps[:, 0:gsz, :],
                        func=AF.Exp)
                    tl += gsz
            # denominator tree for the quarter (l1-l4 DVE bf16 2x, tail
            # Pool mid-kernel / DVE for the latency-critical last quarter).
            # l1 runs in two pieces so most of it overlaps the last exp.
            sl = slice(t0q, t0q + TQ)
            sla = slice(t0q, t0q + GRP[0] + GRP[1])
            slb = slice(t0q + GRP[0] + GRP[1], t0q + TQ)
            nc.vector.tensor_add(tr_t[:, sla, 0:64], p_t[:, sla, 0:64],
                                 p_t[:, sla, 64:128])
            nc.vector.tensor_add(tr_t[:, slb, 0:64], p_t[:, slb, 0:64],
                                 p_t[:, slb, 64:128])
            nc.vector.tensor_add(tr_t[:, sl, 0:32], tr_t[:, sl, 0:32],
                                 tr_t[:, sl, 32:64])
            nc.vector.tensor_add(tr_t[:, sl, 0:16], tr_t[:, sl, 0:16],
                                 tr_t[:, sl, 16:32])
            nc.vector.tensor_add(tr_t[:, sl, 0:8], tr_t[:, sl, 0:8],
                                 tr_t[:, sl, 8:16])
            tail_eng = nc.vector if (ce == CE - 1 and qr == Q - 1) else nc.gpsimd
            tail_eng.tensor_add(tr_t[:, sl, 0:4], tr_t[:, sl, 0:4],
                                tr_t[:, sl, 4:8])
            tail_eng.tensor_add(tr_t[:, sl, 0:2], tr_t[:, sl, 0:2],
                                tr_t[:, sl, 2:4])
            tail_eng.tensor_add(den16[:, ce, sl], tr_t[:, sl, 0:1],
                                tr_t[:, sl, 1:2])

    def emit_post(ce, half=None):
        # r = 1/denom in fp32, bf16 copy for the wsum matmul rhs
        sl = slice(0, TD) if half is None else slice(half * 64, half * 64 + 64)
        nc.vector.tensor_copy(den32[:, ce, sl], den16[:, ce, sl])
        nc.vector.reciprocal_approx_fast(r32[:, ce, sl], den32[:, ce, sl])
        nc.vector.tensor_copy(r16[:, ce, sl], r32[:, ce, sl])

    def emit_wsum(ce, half=None):
        # wsum_T[s, t] += p[e, t, :].T @ r[e, t] (N=1 matmuls, ~free)
        p_t = p_ts[ce]
        sl = slice(0, TD) if half is None else slice(half * 64, half * 64 + 64)
        for t in range(sl.start, sl.stop):
            nc.tensor.matmul(
                wps[:, t:t + 1],
                lhsT=p_t[:, t, :],
                rhs=r16[:, ce, t:t + 1],
                start=True, stop=True,
            )
        if ce == 0:
            nc.vector.tensor_copy(ws_acc[:, sl], wps[:, sl])
        else:
            nc.vector.tensor_add(ws_acc[:, sl], ws_acc[:, sl], wps[:, sl])

    wsumT = singles.tile([P, TD], _F16)
    out_sb = singles.tile([P, D], _F32)

    def emit_final(half):
        t0 = half * 64
        sl = slice(t0, t0 + 64)
        nc.vector.tensor_copy(wsumT[:, sl], ws_acc[:, sl])
        nc.tensor.matmul(ops[t0:t0 + 64, :], lhsT=wsumT[:, sl],
                         rhs=ei_sb[:], start=True, stop=True,
                         tile_position=(0, t0))
        nc.vector.tensor_copy(out_sb[t0:t0 + 64, :], ops[t0:t0 + 64, :])
        nc.sync.dma_start(out=out_ap[t0:t0 + 64, :], in_=out_sb[t0:t0 + 64, :])

    # ---- software-pipelined emission: PE runs scores ahead; wsum(ce) sits
    # two ce behind so it never stalls the PE in front of ACT's exps.  The
    # tail (ce3) interleaves the remaining posts into its tree quarters.
    emit_bfill1(1, 2)
    emit_bfill1(1, 3)
    emit_scores_exp_tree(0)
    for q in range(Q):
        emit_bfill1(2, q)
    emit_scores_exp_tree(1)
    emit_post(0)
    for q in range(Q):
        emit_bfill1(3, q)
    emit_scores_exp_tree(2)
    emit_post(1)
    emit_wsum(0)
    emit_wsum(1)
    emit_scores_exp_tree(3, hooks={
        2: [lambda: emit_post(2)],
        3: [lambda: emit_post(3, 0)],
    })
    emit_wsum(2)
    emit_wsum(3, 0)
    emit_final(0)
    emit_post(3, 1)
    emit_wsum(3, 1)
    emit_final(1)


def build_program():
    if "nc" in _CACHE:
        return _CACHE["nc"]
    nc = bacc.Bacc("TRN2", target_bir_lowering=False, debug=False, num_devices=B)
    wt = nc.dram_tensor("wt", [H, E], _F16, kind="ExternalInput").ap()
    dt = nc.dram_tensor("dt", [H, TD], _F16, kind="ExternalInput").ap()
    et = nc.dram_tensor("et", [E, TE], _F16, kind="ExternalInput").ap()
    ei = nc.dram_tensor("ei", [TE, D], _F16, kind="ExternalInput").ap()
    bd = nc.dram_tensor("bd", [CE * Q * SLAB], _F16, kind="ExternalInput").ap()
    out = nc.dram_tensor("out", [TD, D], _F32, kind="ExternalOutput").ap()
    with tile.TileContext(nc) as tc:
        with ExitStack() as ctx:
            _kernel_body(ctx, tc, out, wt, dt, et, ei, bd)
    nc.compile()
    _CACHE["nc"] = nc
    return nc


_BD_ZEROS = None


def make_in_maps(encoder_inputs, encoder_states, decoder_states, W):
    global _BD_ZEROS
    if _BD_ZEROS is None:
        _BD_ZEROS = np.zeros(CE * Q * SLAB, dtype=np.float16)
    wt_np = np.ascontiguousarray(W.T).astype(np.float16)          # (H, E)
    in_maps = []
    for b in range(B):
        in_maps.append({
            "wt": wt_np,
            "dt": np.ascontiguousarray(decoder_states[:, b, :].T).astype(np.float16),
            "et": np.ascontiguousarray(encoder_states[:, b, :].T).astype(np.float16),
            "ei": np.ascontiguousarray(encoder_inputs[:, b, :]).astype(np.float16),
            "bd": _BD_ZEROS,
        })
    return in_maps


def run_on_hw(in_maps, **kwargs):
    nc = build_program()
    return run_bass_kernel_spmd(nc, in_maps, list(range(B)), **kwargs)


def kernel(**inputs):
    encoder_inputs = np.asarray(inputs["encoder_inputs"], dtype=np.float32)
    encoder_states = np.asarray(inputs["encoder_states"], dtype=np.float32)
    decoder_states = np.asarray(inputs["decoder_states"], dtype=np.float32)
    W = np.asarray(inputs["W"], dtype=np.float32)
    in_maps = make_in_maps(encoder_inputs, encoder_states, decoder_states, W)
    res = run_on_hw(in_maps)
    out = np.stack([res.results[b]["out"] for b in range(B)], axis=1)
    return np.ascontiguousarray(out.astype(np.float32))
